# revision 17
# baseline (speedup 1.0000x reference)
"""Causal self-attention Trainium2 kernel (8 NeuronCores, fp16/fp8 compute).

Sharding: core c -> batch b = c//4, head group hg = c%4 (4 heads each).
Each core computes its heads' QKV projections, causal attention, and a
partial output projection yt[d, t] (transposed, fp16). Host sums the 4
partials per batch, transposes, and adds b_proj.

Dtypes: x is fp8-e3m4 (halves the startup-critical DMA bytes; the PE
streams fp8 moving data at full rate against fp16 weights); all other
16-bit tensors are fp16 (same throughput as bf16 everywhere, 8x finer
mantissa, which pays for the fp8-x quantization in the error budget).

Device dataflow per core:
  startup : ~28 zero matmuls warm the PE clock (HAM) while the first
            DMAs land. Aggregate DMA inflow is ~240 GB/s shared across
            queues, so transfers are ordered strictly by first use
            (w0_0 + span-0 x chunks, w0_1/2, remaining x, w1-3, wproj)
            across the two HW-DGE queues (sync + scalar).
  head 0  : QKV consumed in DMA-arrival order (span 0 group-major, then
            chunk-major across Q/K/V with 3 open PSUM accumulators);
            junk pad-matmuls bridge the残 DMA stalls to keep HAM at 8/8.
  per head: QT/KT/VT = W.T @ x chunks (transposed projections, hd on
            partitions); psum->sbuf copies (+bias) on the Vector engine.
            V = PE-transpose(VT) in two batches of 8 (one PSUM bank).
            per q-span (512): for each k-block kj:
               ST[k,q] = KT_blk.T @ QT_span   (PSUM, f32)
               PT = exp(scale*ST)             (ACT, fp16, unnormalized)
               diag blocks: PT *= tri         (DVE, multiplicative mask)
               acc += PT                      (DVE, fp16, 2x mode)
               OT[hd,q] += V_blk.T @ PT       (PE)
            span end: sums = ones16.T @ acc (PE, one matmul broadcasts
            the partition reduction over hd), recipT = 1/sums (DVE),
            OT_sbuf = OT * recipT (DVE, fp16)
  pipeline: next head's QKV matmuls are queued as thunks and injected
            one per attention block, so the exp-gated attention phase
            keeps the Tensor engine busy; the remainder drains between
            heads. During the last head, early proj stripes inject.
  proj    : yt[dc, t] += Wp_blk.T @ OT_h over heads; evictions
            alternate DVE/ACT and the yt DMAs alternate queues; the
            drain rotates over the freed score/pv/sum PSUM banks; the
            final stripe evicts in two parallel halves.
PSUM banks: qkv 2, pv 1, scores 3, transpose 1, sums 1 = 8.
"""
import numpy as np
from collections import deque

B, S, D, H = 2, 2048, 2048, 16
HD = 128
NCORES = 8
HPC = H // (NCORES // B)     # heads per core = 4
WARMUP = 28


def build_nc(S=S, D=D, nh=HPC, span=512):
    import concourse.bass as bass
    import concourse.mybir as mybir
    from concourse import bacc
    from concourse.tile import TileContext

    f32 = mybir.dt.float32
    f16 = mybir.dt.float16
    bf16 = mybir.dt.bfloat16
    KT = D // 128          # contraction tiles for qkv
    TT = S // 128          # token tiles
    NS = S // span         # q spans
    KPS = span // 128      # k-blocks per span
    SPW = KT * span        # packed x columns per span
    scale = float(HD) ** -0.5

    nc = bacc.Bacc("TRN2", target_bir_lowering=False, debug=False)
    f8 = mybir.dt.float8e3
    x_d = nc.dram_tensor("xt", [128, NS * SPW], f8, kind="ExternalInput").ap()
    wq_d = nc.dram_tensor("wqkv", [3 * nh * 128, D], f16, kind="ExternalInput").ap()
    bq_d = nc.dram_tensor("bqkv", [128, 3 * nh], f32, kind="ExternalInput").ap()
    wp_d = nc.dram_tensor("wproj", [nh * 128, D], f16, kind="ExternalInput").ap()
    tb_d = nc.dram_tensor("tribin", [128, 128], f16, kind="ExternalInput").ap()
    id_d = nc.dram_tensor("identb", [128, 128], f16, kind="ExternalInput").ap()
    on_d = nc.dram_tensor("ones16", [128, 128], f16, kind="ExternalInput").ap()
    yt_d = nc.dram_tensor("yt", [D, S], f16, kind="ExternalOutput").ap()

    Act = mybir.ActivationFunctionType
    Alu = mybir.AluOpType

    with TileContext(nc) as tc:
        from contextlib import ExitStack
        with ExitStack() as ctx:
            res = ctx.enter_context(tc.tile_pool(name="res", bufs=1))
            w_p = ctx.enter_context(tc.tile_pool(name="w", bufs=1))
            wp_p = ctx.enter_context(tc.tile_pool(name="wp", bufs=1))
            qk_p = ctx.enter_context(tc.tile_pool(name="qk", bufs=2))
            v_p = ctx.enter_context(tc.tile_pool(name="v", bufs=2))
            pt_p = ctx.enter_context(tc.tile_pool(name="pt", bufs=8))
            acc_p = ctx.enter_context(tc.tile_pool(name="acc", bufs=2))
            sm_p = ctx.enter_context(tc.tile_pool(name="sm", bufs=2))
            yst_p = ctx.enter_context(tc.tile_pool(name="yst", bufs=6))
            ps_qkv = ctx.enter_context(tc.tile_pool(name="ps_qkv", bufs=2, space="PSUM"))
            ps_pv = ctx.enter_context(tc.tile_pool(name="ps_pv", bufs=1, space="PSUM"))
            ps_t = ctx.enter_context(tc.tile_pool(name="ps_t", bufs=1, space="PSUM"))
            ps_st = ctx.enter_context(tc.tile_pool(name="ps_st", bufs=3, space="PSUM"))
            ps_sm = ctx.enter_context(tc.tile_pool(name="ps_sm", bufs=1, space="PSUM"))

            # ---- PE warmup: zero matmuls (no DMA deps) ramp the HAM
            # clock while the first input DMAs are in flight. ----
            warm = res.tile([128, 128], bf16, tag="warm")
            nc.vector.memset(warm, 0.0)
            wps = ps_st.tile([128, span], f32, tag="st")
            for _ in range(WARMUP):
                nc.tensor.matmul(wps[:, :128], warm, warm, start=True, stop=True)

            # ---- startup DMAs split over 4 queues (issuing engine = queue):
            #   sync/vector: x (span 0 in kt chunks, then span halves)
            #   scalar     : qkv weights    gpsimd: bias, consts, wproj ----
            wstrip = {}

            # x rides the two HW-DGE queues (sync + scalar); the slow
            # SWDGE gpsimd queue only carries small/late tensors.
            xall = res.tile([128, NS * SPW], f8, tag="xall")
            bq = res.tile([128, 3 * nh], f32, tag="bq")
            tribin = res.tile([128, 128], f16, tag="tribin")
            identb = res.tile([128, 128], f16, tag="identb")
            ones16 = res.tile([128, 128], f16, tag="ones16")
            nc.gpsimd.dma_start(bq, bq_d)
            nc.gpsimd.dma_start(tribin, tb_d)
            nc.gpsimd.dma_start(identb, id_d)
            nc.gpsimd.dma_start(ones16, on_d)
            # Aggregate DMA inflow is ~240 GB/s shared across all queues,
            # so order strictly by first-use: w0_0 + span0 x, w0_1/w0_2,
            # then remaining x, then later heads' weights, wproj last.
            nchunk = min(4, KT)
            ktper = KT // nchunk

            def xchunk(eng, sp, c):
                lo = sp * SPW + c * ktper * span
                hi = sp * SPW + (c + 1) * ktper * span
                eng.dma_start(xall[:, lo:hi], x_d[:, lo:hi])

            def w_dma_on(eng, h, p):
                wt = w_p.tile([128, D], f16, tag=f"w{h}_{p}", name=f"w{h}_{p}")
                eng.dma_start(
                    wt, wq_d[(p * nh + h) * 128:(p * nh + h + 1) * 128, :])
                wstrip[(h, p)] = wt

            w00 = w_p.tile([128, D], f16, tag="w0_0", name="w0_0")
            nc.sync.dma_start(w00[:, :D // 2], wq_d[0:128, :D // 2])
            nc.scalar.dma_start(w00[:, D // 2:], wq_d[0:128, D // 2:])
            wstrip[(0, 0)] = w00
            xchunk(nc.sync, 0, 0)
            if nchunk > 1:
                xchunk(nc.scalar, 0, 1)
            for c in range(2, nchunk):
                xchunk(nc.sync, 0, c)
            w_dma_on(nc.scalar, 0, 1)
            w_dma_on(nc.scalar, 0, 2)
            for sp in range(1, NS):
                for c in range(nchunk):
                    xchunk(nc.sync if c % 2 == 0 else nc.scalar, sp, c)
            for h in range(1, nh):
                w_dma_on(nc.sync, h, 0)
                w_dma_on(nc.scalar, h, 1)
                w_dma_on(nc.sync, h, 2)
            wp = []
            for g in range(nh):
                w = wp_p.tile([128, D], f16, tag=f"wpt{g}")
                (nc.sync if g % 2 == 0 else nc.scalar).dma_start(
                    w, wp_d[g * 128:(g + 1) * 128, :])
                wp.append(w)

            # ---- per-head OT accumulation ----
            OT = [res.tile([128, S], f16, tag=f"ot{h}", name=f"ot{h}")
                  for h in range(nh)]

            # ---- deferred-PE work queue: QKV matmuls of the next head
            # (and early proj stripes during the last head) are emitted
            # one per attention block to fill exp-gated PE idle time. ----
            work_q = deque()

            def pull(n):
                for _ in range(min(n, len(work_q))):
                    work_q.popleft()()

            def drain_q():
                while work_q:
                    work_q.popleft()()

            def enqueue_qkv(h):
                wts = [wstrip.pop((h, p)) for p in range(3)]
                qt = qk_p.tile([128, S], f16, tag="qt", name=f"qt{h}")
                kt_ = qk_p.tile([128, S], f16, tag="kt_", name=f"kt_{h}")
                vt = qk_p.tile([128, S], f16, tag="vt", name=f"vt{h}")
                qkvT = [qt, kt_, vt]
                for sp in range(NS):
                    for p in range(3):
                        cell = {}
                        for kt in range(KT):
                            def mm(kt=kt, sp=sp, wt=wts[p], cell=cell):
                                if kt == 0:
                                    cell['ps'] = ps_qkv.tile(
                                        [128, span], f32, tag="qkv",
                                        name=f"qg{sp}")
                                nc.tensor.matmul(
                                    cell['ps'], wt[:, kt * 128:(kt + 1) * 128],
                                    xall[:, sp * SPW + kt * span:
                                         sp * SPW + (kt + 1) * span],
                                    start=(kt == 0), stop=(kt == KT - 1))
                            work_q.append(mm)

                        def ev(sp=sp, hp=p * nh + h, cell=cell, dst=qkvT[p]):
                            nc.vector.tensor_scalar(
                                out=dst[:, sp * span:(sp + 1) * span],
                                in0=cell['ps'], scalar1=bq[:, hp:hp + 1],
                                scalar2=None, op0=Alu.add)
                        work_q.append(ev)
                return qkvT

            # proj: one stripe = 4 head-matmuls + cast + dma for (dc, sp)
            proj_done = set()

            def proj_stripe(dc, sp, thunks, pool=None, ptag="qkv", eng=None):
                cell = {}
                if pool is None:
                    pool = ps_qkv
                for g in range(nh):
                    def mm(g=g, dc=dc, sp=sp, cell=cell, pool=pool, ptag=ptag):
                        if g == 0:
                            cell['ps'] = pool.tile([128, span], f32,
                                                   tag=ptag, name=f"pj{dc}_{sp}")
                        nc.tensor.matmul(
                            cell['ps'], wp[g][:, dc * 128:(dc + 1) * 128],
                            OT[g][:, sp * span:(sp + 1) * span],
                            start=(g == 0), stop=(g == nh - 1))
                    thunks.append(mm)

                def ev(dc=dc, sp=sp, cell=cell, eng=eng):
                    yst = yst_p.tile([128, span], f16, tag="yst",
                                     name=f"yst{dc}_{sp}")
                    if eng is None:
                        nc.vector.tensor_copy(yst, cell['ps'])
                    else:
                        eng.copy(yst, cell['ps'])
                    outq = (nc.sync if (eng is None or (dc + sp) % 2 == 0)
                            else nc.scalar)
                    outq.dma_start(yt_d[dc * 128:(dc + 1) * 128,
                                        sp * span:(sp + 1) * span], yst)
                thunks.append(ev)
                proj_done.add((dc, sp))

            def emit_qkv0():
                # head 0 runs while x is still streaming in: consume
                # chunk-major across the three projections so PE demand
                # follows DMA arrival order (3 open PSUM accumulators).
                wts = [wstrip.pop((0, p)) for p in range(3)]
                qt = qk_p.tile([128, S], f16, tag="qt", name="qt0")
                kt_ = qk_p.tile([128, S], f16, tag="kt_", name="kt_0")
                vt = qk_p.tile([128, S], f16, tag="vt", name="vt0")
                qkvT = [qt, kt_, vt]
                for sp in range(NS):
                    cells = [
                        ps_qkv.tile([128, span], f32, tag="qkv", name=f"h0q{sp}"),
                        ps_qkv.tile([128, span], f32, tag="qkv", name=f"h0k{sp}"),
                        ps_pv.tile([128, span], f32, tag="pv", name=f"h0v{sp}"),
                    ]
                    if sp == 0:
                        # group-major: only w0_0 + span-0 x needed at start
                        order = [(p, kt) for p in range(3) for kt in range(KT)]
                    else:
                        order = [(p, kt) for c in range(nchunk) for p in range(3)
                                 for kt in range(c * ktper, (c + 1) * ktper)]
                    pad_after = {(0, KT // 4 - 1): 4,
                                 (0, KT - 1): 2} if sp == 0 else {}
                    padn = [0]
                    for p, kt in order:
                        nc.tensor.matmul(
                            cells[p], wts[p][:, kt * 128:(kt + 1) * 128],
                            xall[:, sp * SPW + kt * span:
                                 sp * SPW + (kt + 1) * span],
                            start=(kt == 0), stop=(kt == KT - 1))
                        if pad_after.get((p, kt)):
                            pt_ = ps_st.tile([128, span], f32, tag="st",
                                             name=f"pad{padn[0]}")
                            padn[0] += 1
                            for _ in range(pad_after[(p, kt)]):
                                nc.tensor.matmul(pt_[:, :128], warm, warm,
                                                 start=True, stop=True)
                    for p in range(3):
                        nc.vector.tensor_scalar(
                            out=qkvT[p][:, sp * span:(sp + 1) * span],
                            in0=cells[p], scalar1=bq[:, p * nh:p * nh + 1],
                            scalar2=None, op0=Alu.add)
                return qkvT

            # ---- head loop ----
            qkvT_next = emit_qkv0()
            for h in range(nh):
                QT, KTt, VT = qkvT_next
                if h + 1 < nh:
                    qkvT_next = enqueue_qkv(h + 1)

                vh = v_p.tile([128, S], f16, tag="v")

                def transpose_batch(tg0, tgn, engs=None):
                    ps = ps_t.tile([128, 1024], f16, tag="tp")
                    for j in range(tgn - tg0):
                        nc.tensor.transpose(
                            ps[:, j * 128:(j + 1) * 128],
                            VT[:, (tg0 + j) * 128:(tg0 + j + 1) * 128], identb)
                    nc.vector.tensor_copy(
                        vh[:, tg0 * 128:tgn * 128], ps[:, :(tgn - tg0) * 128])

                transpose_batch(0, min(8, TT))

                for sp in range(NS):
                    if sp == 2 and TT > 8:
                        transpose_batch(8, TT)
                    nkj = KPS * (sp + 1)   # causal: k-blocks 0..nkj-1
                    ps_o = ps_pv.tile([128, span], f32, tag="pv")
                    acc = acc_p.tile([128, span], f16, tag="acc")
                    pend = []  # (kj, pt, qoff) awaiting PV emission

                    def flush_one(nkj=None, ps_o=None):
                        kj, pt, qoff = pend.pop(0)
                        nc.tensor.matmul(
                            ps_o[:, qoff:], vh[:, kj * 128:(kj + 1) * 128],
                            pt[:, qoff:], start=(kj == 0), stop=(kj == nkj - 1))

                    for kj in range(nkj):
                        qoff = max(0, (kj - KPS * sp)) * 128
                        st = ps_st.tile([128, span], f32, tag="st")
                        nc.tensor.matmul(
                            st[:, qoff:], KTt[:, kj * 128:(kj + 1) * 128],
                            QT[:, sp * span + qoff:(sp + 1) * span],
                            start=True, stop=True)
                        pull(1)
                        pt = pt_p.tile([128, span], f16, tag="pt")
                        nc.scalar.activation(
                            pt[:, qoff:], st[:, qoff:], Act.Exp, scale=scale)
                        if kj >= KPS * sp:  # diagonal block: causal mask
                            nc.vector.tensor_tensor(
                                out=pt[:, qoff:qoff + 128],
                                in0=pt[:, qoff:qoff + 128],
                                in1=tribin, op=Alu.mult)
                        if kj == 0:
                            nc.vector.tensor_copy(acc, pt)
                        else:
                            nc.vector.tensor_tensor(
                                out=acc[:, qoff:], in0=acc[:, qoff:],
                                in1=pt[:, qoff:], op=Alu.add)
                        pend.append((kj, pt, qoff))
                        if len(pend) > 5:
                            flush_one(nkj=nkj, ps_o=ps_o)
                    while pend:
                        flush_one(nkj=nkj, ps_o=ps_o)
                    pull(4)

                    ps_s = ps_sm.tile([128, span], f32, tag="sum")
                    nc.tensor.matmul(ps_s, ones16, acc, start=True, stop=True)
                    recipb = sm_p.tile([128, span], f32, tag="recipb")
                    nc.vector.reciprocal_approx_fast(out=recipb, in_=ps_s)
                    nc.vector.tensor_tensor(
                        out=OT[h][:, sp * span:(sp + 1) * span],
                        in0=ps_o, in1=recipb, op=Alu.mult)

                    if h == nh - 1:
                        # inject early proj stripes (bounded by DVE slack
                        # and by pulls remaining, to avoid leftovers that
                        # drain back-to-back on 2 PSUM bufs)
                        if sp == 0:
                            for dc in range(min(6, D // 128)):
                                proj_stripe(dc, 0, work_q)
                        elif sp == 1:
                            for dc in range(6, min(8, D // 128)):
                                proj_stripe(dc, 0, work_q)

                drain_q()

            # ---- remaining output projection (rotating over the now-free
            # score/pv PSUM banks for a deeper eviction pipeline) ----
            rot = [(ps_qkv, "qkv"), (ps_st, "st"), (ps_pv, "pv"),
                   (ps_qkv, "qkv"), (ps_st, "st"), (ps_sm, "sum")]
            ri = 0
            todo = [(dc, sp) for dc in range(D // 128) for sp in range(NS)
                    if (dc, sp) not in proj_done]
            for dc, sp in (todo[:-1] if todo else []):
                thunks = []
                pool, ptag = rot[ri % len(rot)]
                eng = nc.scalar if ri % 2 else None
                ri += 1
                proj_stripe(dc, sp, thunks, pool=pool, ptag=ptag, eng=eng)
                for t in thunks:
                    t()
            # last stripe: evict in two halves on both engines/queues so
            # the tail chain after the final matmul is ~halved
            dc, sp = todo[-1] if todo else (None, None)
            cell = {}
            for g in range(nh if todo else 0):
                if g == 0:
                    cell['ps'] = ps_st.tile([128, span], f32, tag="st",
                                            name="pj_last")
                nc.tensor.matmul(
                    cell['ps'], wp[g][:, dc * 128:(dc + 1) * 128],
                    OT[g][:, sp * span:(sp + 1) * span],
                    start=(g == 0), stop=(g == nh - 1))
            h1 = span // 2
            if not todo:
                h1 = None
            ya = None if h1 is None else yst_p.tile(
                [128, h1], f16, tag="ylast_a", name="ylast_a")
            if h1 is not None:
                yb = yst_p.tile([128, h1], f16, tag="ylast_b", name="ylast_b")
                nc.vector.tensor_copy(ya, cell['ps'][:, :h1])
                nc.scalar.copy(yb, cell['ps'][:, h1:])
                nc.sync.dma_start(
                    yt_d[dc * 128:(dc + 1) * 128,
                         sp * span:sp * span + h1], ya)
                nc.scalar.dma_start(
                    yt_d[dc * 128:(dc + 1) * 128,
                         sp * span + h1:(sp + 1) * span], yb)

    nc.finalize()
    return nc


def pack_x(xb, S=S, D=D, span=512):
    """[S, D] -> span-major packed [128, NS*KT*span] (bf16 upstream)."""
    NS, KT = S // span, D // 128
    return np.ascontiguousarray(
        xb.reshape(NS, span, KT, 128).transpose(3, 0, 2, 1).reshape(128, -1))


def _prep_core_inputs(x, W_qkv, b_qkv, W_proj, core, S=S, D=D, nh=HPC):
    import ml_dtypes
    f8e3 = ml_dtypes.float8_e3m4
    f16 = np.float16
    ngr = NCORES // B
    b, hg = core // ngr, core % ngr
    KT = D // 128
    Dfull = W_qkv.shape[0]

    wq = np.empty((3 * nh * 128, D), dtype=f16)
    bq = np.zeros((128, 3 * nh), dtype=np.float32)
    for p in range(3):
        for h in range(nh):
            g = hg * nh + h
            col = p * Dfull + g * 128
            blk = W_qkv[:, col:col + 128]            # [D, 128]
            hp = p * nh + h
            wq[hp * 128:(hp + 1) * 128] = (
                blk.reshape(KT, 128, 128).transpose(1, 0, 2).reshape(128, D)
                .astype(f16))
            bq[:, hp] = b_qkv[col:col + 128]
    wp = W_proj[hg * nh * 128:(hg + 1) * nh * 128, :].astype(f16)

    r = np.arange(128)
    tribin = np.where(r[:, None] <= r[None, :], 1.0, 0.0)
    return {
        "xt": pack_x(x[b]).astype(f8e3),
        "wqkv": wq,
        "bqkv": bq,
        "wproj": wp,
        "tribin": tribin.astype(f16),
        "identb": np.eye(128, dtype=f16),
        "ones16": np.ones((128, 128), dtype=np.float16),
    }


_CACHE = {}


def kernel(x, W_qkv, b_qkv, W_proj, b_proj, mask):
    from concourse.bass_utils import run_bass_kernel_spmd

    x = np.asarray(x)
    W_qkv = np.asarray(W_qkv)
    b_qkv = np.asarray(b_qkv)
    W_proj = np.asarray(W_proj)
    b_proj = np.asarray(b_proj)

    if "nc" not in _CACHE:
        _CACHE["nc"] = build_nc()
    nc = _CACHE["nc"]

    in_maps = [_prep_core_inputs(x, W_qkv, b_qkv, W_proj, c)
               for c in range(NCORES)]
    res = run_bass_kernel_spmd(nc, in_maps, core_ids=list(range(NCORES)))

    ngr = NCORES // B
    out = np.empty((B, S, D), dtype=np.float32)
    for b in range(B):
        acc = res.results[b * ngr]["yt"].astype(np.float32)
        for g in range(1, ngr):
            acc = acc + res.results[b * ngr + g]["yt"]
        out[b] = acc.T + b_proj[None, :]
    return out
